# revision 5
# baseline (speedup 1.0000x reference)
"""Trainium2 Bass kernel for a KAN layer.

out[i] = sum_{j,k} B[j,k] * coeffs[j,i,k] + sum_j silu(x[j]) * base_weights[j,i]

where B is the degree-3 B-spline basis (10 uniform knots on [-1,1] -> 6 basis
functions) evaluated at x[j].  j in [0,4096), i in [0,2048), k in [0,6).

Strategy (8 NeuronCores, tensor-parallel over out_feat):
  - Each core owns a 256-wide slice of out_feat: coeffs[:, c*256:(c+1)*256, :]
    (24 MiB) and base_weights[:, c*256:(c+1)*256] (4 MiB).
  - The tiny B-spline basis B [4096,6] and silu(x) [4096] are precomputed on
    host (they depend only on the 16 KiB input x) and packed per 128-row
    j-chunk into a [128, 32*7] stationary matrix.
  - On device, per 128-row j-chunk: DMA the coeffs chunk [128, 256, 6]
    contiguously, then 6 matmuls (one per basis k) with lhsT = B[:,k] (a
    [128,1] column) and rhs = the stride-6 slice chunk[:, :, k], plus 1 matmul
    for the silu*base_weights term -- all 224 matmuls accumulate into a single
    PSUM [1, 256] tile.  The reduction over j and k happens entirely inside
    the PE array / PSUM; memory traffic is the roofline term (28 MiB/core).
  - Matmul operands are bitcast to float32r (full-rate fp32 matmul mode).
"""

import numpy as np

IN_FEAT = 4096
OUT_FEAT = 2048
NB = 6  # number of B-spline basis functions
N_CORES = 8
ISH = OUT_FEAT // N_CORES  # 256 out features per core
P = 128  # SBUF partitions
NCHUNK = IN_FEAT // P  # 32 j-chunks
GRID_MIN, GRID_MAX = -1.0, 1.0
NUM_KNOTS = 10
DEGREE = 3

MM_DTYPE = "float32r"  # "float32r" (full-rate) or "float32" (4x slower, exact)
COF_BUFS = 4
BW_BUFS = 4


def _bspline_basis(x):
    """Cox-de Boor, mirrors reference.bspline_basis in fp32 numpy."""
    t = np.linspace(GRID_MIN, GRID_MAX, NUM_KNOTS, dtype=np.float32)
    xe = x[:, None].astype(np.float32)
    N = ((xe >= t[:-1]) & (xe < t[1:])).astype(np.float32)
    for d in range(1, DEGREE + 1):
        left_den = t[d:-1] - t[: -d - 1]
        right_den = t[d + 1 :] - t[1:-d]
        left = (
            np.where(
                left_den > 0, (xe - t[: -d - 1]) / np.where(left_den > 0, left_den, 1.0), 0.0
            )
            * N[:, :-1]
        )
        right = (
            np.where(
                right_den > 0, (t[d + 1 :] - xe) / np.where(right_den > 0, right_den, 1.0), 0.0
            )
            * N[:, 1:]
        )
        N = (left + right).astype(np.float32)
    return N  # [J, 6]


def build_bass():
    """Build the per-core Bass program (identical on all 8 cores)."""
    import concourse.tile as tile
    from concourse import bacc, mybir

    f32 = mybir.dt.float32
    mm_dt = getattr(mybir.dt, MM_DTYPE)

    nc = bacc.Bacc("TRN2", target_bir_lowering=False, debug=False, enable_asserts=False)
    cof = nc.dram_tensor("cof", [IN_FEAT, ISH, NB], mm_dt, kind="ExternalInput").ap()
    bw = nc.dram_tensor("bw", [IN_FEAT, ISH], mm_dt, kind="ExternalInput").ap()
    # bsx[p, jc*7 + k] = B[jc*128+p, k] for k<6 ; = silu(x[jc*128+p]) for k==6
    bsx = nc.dram_tensor("bsx", [P, NCHUNK * 7], mm_dt, kind="ExternalInput").ap()
    out = nc.dram_tensor("out", [1, ISH], f32, kind="ExternalOutput").ap()

    with tile.TileContext(nc) as tc:
        with (
            tc.tile_pool(name="const", bufs=1) as constp,
            tc.tile_pool(name="cofp", bufs=COF_BUFS) as cofp,
            tc.tile_pool(name="bwp", bufs=BW_BUFS) as bwp,
            tc.tile_pool(name="outp", bufs=1) as outp,
            tc.tile_pool(name="psum", bufs=1, space="PSUM") as psp,
        ):
            bsx_t = constp.tile([P, NCHUNK * 7], mm_dt)
            nc.sync.dma_start(bsx_t[:], bsx[:])
            acc = psp.tile([1, ISH], f32)
            for jc in range(NCHUNK):
                ct = cofp.tile([P, ISH, NB], mm_dt)
                nc.sync.dma_start(ct[:], cof[jc * P : (jc + 1) * P])
                bt = bwp.tile([P, ISH], mm_dt)
                nc.sync.dma_start(bt[:], bw[jc * P : (jc + 1) * P])
                for k in range(NB):
                    nc.tensor.matmul(
                        acc[:],
                        bsx_t[:, jc * 7 + k : jc * 7 + k + 1],
                        ct[:, :, k],
                        start=(jc == 0 and k == 0),
                        stop=False,
                    )
                nc.tensor.matmul(
                    acc[:],
                    bsx_t[:, jc * 7 + 6 : jc * 7 + 7],
                    bt[:],
                    start=False,
                    stop=(jc == NCHUNK - 1),
                )
            ot = outp.tile([1, ISH], f32)
            nc.vector.tensor_copy(ot[:], acc[:])
            nc.sync.dma_start(out[:], ot[:])
    nc.compile()
    return nc


_STATE = None


def _build_state():
    global _STATE
    if _STATE is not None:
        return _STATE

    import jax
    from jax.experimental.shard_map import shard_map
    from jax.sharding import Mesh, PartitionSpec
    from concourse import bass2jax, mybir

    nc = build_bass()

    partition_name = nc.partition_id_tensor.name if nc.partition_id_tensor else None
    in_names, out_names, out_avals, zero_outs = [], [], [], []
    for alloc in nc.m.functions[0].allocations:
        if not isinstance(alloc, mybir.MemoryLocationSet):
            continue
        name = alloc.memorylocations[0].name
        if alloc.kind == "ExternalInput":
            if name == partition_name:
                continue
            in_names.append(name)
        elif alloc.kind == "ExternalOutput":
            out_names.append(name)
            shape = tuple(alloc.tensor_shape)
            dtp = mybir.dt.np(alloc.dtype)
            out_avals.append(jax.core.ShapedArray(shape, dtp))
            zero_outs.append(np.zeros(shape, dtp))
    n_params = len(in_names)
    all_in_names = tuple(in_names) + tuple(out_names)
    if partition_name is not None:
        all_in_names = all_in_names + (partition_name,)

    bass2jax.install_neuronx_cc_hook()
    devices = jax.devices()[:N_CORES]
    mesh = Mesh(np.asarray(devices), ("core",))

    def _body(*args):
        operands = list(args)
        if partition_name is not None:
            operands.append(bass2jax.partition_id_tensor())
        outs = bass2jax._bass_exec_p.bind(
            *operands,
            out_avals=tuple(out_avals),
            in_names=all_in_names,
            out_names=tuple(out_names),
            lowering_input_output_aliases=(),
            sim_require_finite=True,
            sim_require_nnan=True,
            nc=nc,
        )
        return tuple(outs)

    in_specs = (PartitionSpec("core"),) * (n_params + len(out_names))
    out_specs = (PartitionSpec("core"),) * len(out_names)
    jfn = jax.jit(
        shard_map(_body, mesh=mesh, in_specs=in_specs, out_specs=out_specs, check_rep=False),
        keep_unused=True,
    )
    _STATE = dict(
        nc=nc,
        jfn=jfn,
        in_names=in_names,
        out_names=out_names,
        zero_outs=zero_outs,
        mesh=mesh,
        pspec=PartitionSpec("core"),
        jax=jax,
    )
    return _STATE


def prepare_global_args(x, coeffs, base_weights):
    """Host prep: basis/silu precompute + global (8*shape[0], ...) concat
    arrays in the order the jitted function expects them."""
    x = np.asarray(x, dtype=np.float32)
    coeffs = np.asarray(coeffs, dtype=np.float32)
    base_weights = np.asarray(base_weights, dtype=np.float32)

    B = _bspline_basis(x)  # [4096, 6]
    sx = (x / (1.0 + np.exp(-x))).astype(np.float32)  # silu
    packed = np.concatenate(
        [B.reshape(NCHUNK, P, NB), sx.reshape(NCHUNK, P, 1)], axis=2
    )  # [32, 128, 7]
    bsx = np.ascontiguousarray(packed.transpose(1, 0, 2).reshape(P, NCHUNK * 7))

    st = _build_state()
    glob = {
        "cof": np.ascontiguousarray(
            coeffs.reshape(IN_FEAT, N_CORES, ISH, NB).transpose(1, 0, 2, 3)
        ).reshape(N_CORES * IN_FEAT, ISH, NB),
        "bw": np.ascontiguousarray(
            base_weights.reshape(IN_FEAT, N_CORES, ISH).transpose(1, 0, 2)
        ).reshape(N_CORES * IN_FEAT, ISH),
        "bsx": np.tile(bsx, (N_CORES, 1)),
    }
    args = [glob[name] for name in st["in_names"]]
    for z in st["zero_outs"]:
        args.append(np.tile(z, (N_CORES,) + (1,) * (z.ndim - 1)))
    return args


def kernel(x, coeffs, base_weights):
    st = _build_state()
    args = prepare_global_args(x, coeffs, base_weights)
    outs = st["jfn"](*args)
    out_g = np.asarray(outs[0])  # [8, 256]
    return out_g.reshape(OUT_FEAT).astype(np.float32)


# revision 9
# speedup vs baseline: 56.6327x; 56.6327x over previous
"""Trainium2 Bass kernel for a KAN layer.

out[i] = sum_{j,k} B[j,k] * coeffs[j,i,k] + sum_j silu(x[j]) * base_weights[j,i]

where B is the degree-3 B-spline basis (10 uniform knots on [-1,1] -> 6 basis
functions) evaluated at x[j].  j in [0,4096), i in [0,2048), k in [0,6).

Strategy (8 NeuronCores, tensor-parallel over out_feat):
  - Each core owns a 256-wide slice of out_feat: coeffs[:, c*256:(c+1)*256, :]
    (24 MiB) and base_weights[:, c*256:(c+1)*256] (4 MiB).
  - The tiny B-spline basis B [4096,6] and silu(x) [4096] are precomputed on
    host (they depend only on the 16 KiB input x) and packed per 128-row
    j-chunk into a [128, 32*7] stationary matrix.
  - On device, per 128-row j-chunk: DMA the coeffs chunk [128, 256, 6]
    contiguously, then 6 matmuls (one per basis k) with lhsT = B[:,k] (a
    [128,1] column) and rhs = the stride-6 slice chunk[:, :, k], plus 1 matmul
    for the silu*base_weights term -- all 224 matmuls accumulate into a single
    PSUM [1, 256] tile.  The reduction over j and k happens entirely inside
    the PE array / PSUM; memory traffic is the roofline term (28 MiB/core).
  - Matmul operands are bitcast to float32r (full-rate fp32 matmul mode).
"""

import numpy as np

IN_FEAT = 4096
OUT_FEAT = 2048
NB = 6  # number of B-spline basis functions
N_CORES = 8
ISH = OUT_FEAT // N_CORES  # 256 out features per core
P = 128  # SBUF partitions
NCHUNK = IN_FEAT // P  # 32 j-chunks
GRID_MIN, GRID_MAX = -1.0, 1.0
NUM_KNOTS = 10
DEGREE = 3

MM_DTYPE = "float32r"  # "float32r" (full-rate) or "float32" (4x slower, exact)
COF_BUFS = 4
BW_BUFS = 4


def _bspline_basis(x):
    """Cox-de Boor, mirrors reference.bspline_basis in fp32 numpy."""
    t = np.linspace(GRID_MIN, GRID_MAX, NUM_KNOTS, dtype=np.float32)
    xe = x[:, None].astype(np.float32)
    N = ((xe >= t[:-1]) & (xe < t[1:])).astype(np.float32)
    for d in range(1, DEGREE + 1):
        left_den = t[d:-1] - t[: -d - 1]
        right_den = t[d + 1 :] - t[1:-d]
        left = (
            np.where(
                left_den > 0, (xe - t[: -d - 1]) / np.where(left_den > 0, left_den, 1.0), 0.0
            )
            * N[:, :-1]
        )
        right = (
            np.where(
                right_den > 0, (t[d + 1 :] - xe) / np.where(right_den > 0, right_den, 1.0), 0.0
            )
            * N[:, 1:]
        )
        N = (left + right).astype(np.float32)
    return N  # [J, 6]


def build_bass(nchunk=NCHUNK, repeats=1, dynamic=False):
    """Build the per-core Bass program (identical on all 8 cores)."""
    import concourse.tile as tile
    from concourse import bacc, mybir

    f32 = mybir.dt.float32
    mm_dt = getattr(mybir.dt, MM_DTYPE)

    nc = bacc.Bacc("TRN2", target_bir_lowering=False, debug=False, enable_asserts=False)
    cof = nc.dram_tensor("cof", [IN_FEAT, ISH, NB], mm_dt, kind="ExternalInput").ap()
    bw = nc.dram_tensor("bw", [IN_FEAT, ISH], mm_dt, kind="ExternalInput").ap()
    # bsx[p, jc*7 + k] = B[jc*128+p, k] for k<6 ; = silu(x[jc*128+p]) for k==6
    bsx = nc.dram_tensor("bsx", [P, NCHUNK * 7], mm_dt, kind="ExternalInput").ap()
    out = nc.dram_tensor("out", [1, ISH], f32, kind="ExternalOutput").ap()

    with tile.TileContext(nc) as tc:
        with (
            tc.tile_pool(name="const", bufs=1) as constp,
            tc.tile_pool(name="cofp", bufs=COF_BUFS) as cofp,
            tc.tile_pool(name="bwp", bufs=BW_BUFS) as bwp,
            tc.tile_pool(name="outp", bufs=1) as outp,
            tc.tile_pool(name="psum", bufs=1, space="PSUM") as psp,
        ):
            bsx_t = constp.tile([P, NCHUNK * 7], mm_dt)
            nc.sync.dma_start(bsx_t[:], bsx[:])
            acc = psp.tile([1, ISH], f32)

            def sweep():
                for jc in range(nchunk):
                    ct = cofp.tile([P, ISH, NB], mm_dt)
                    nc.sync.dma_start(ct[:], cof[jc * P : (jc + 1) * P])
                    bt = bwp.tile([P, ISH], mm_dt)
                    nc.sync.dma_start(bt[:], bw[jc * P : (jc + 1) * P])
                    for k in range(NB):
                        nc.tensor.matmul(
                            acc[:],
                            bsx_t[:, jc * 7 + k : jc * 7 + k + 1],
                            ct[:, :, k],
                            start=(jc == 0 and k == 0),
                            stop=False,
                        )
                    nc.tensor.matmul(
                        acc[:],
                        bsx_t[:, jc * 7 + 6 : jc * 7 + 7],
                        bt[:],
                        start=False,
                        stop=(jc == nchunk - 1),
                    )

            if dynamic and repeats > 1:
                with tc.For_i(0, repeats, 1):
                    sweep()
            else:
                for _ in range(repeats):
                    sweep()
            ot = outp.tile([1, ISH], f32)
            nc.vector.tensor_copy(ot[:], acc[:])
            nc.sync.dma_start(out[:], ot[:])
    nc.compile()
    return nc


_STATE = {}


def _build_state(nchunk=NCHUNK, repeats=1, dynamic=False):
    key = (nchunk, repeats, dynamic)
    if key in _STATE:
        return _STATE[key]

    import jax
    from jax.experimental.shard_map import shard_map
    from jax.sharding import Mesh, PartitionSpec
    from concourse import bass2jax, mybir

    nc = build_bass(nchunk, repeats, dynamic)

    partition_name = nc.partition_id_tensor.name if nc.partition_id_tensor else None
    in_names, out_names, out_avals, zero_outs = [], [], [], []
    for alloc in nc.m.functions[0].allocations:
        if not isinstance(alloc, mybir.MemoryLocationSet):
            continue
        name = alloc.memorylocations[0].name
        if alloc.kind == "ExternalInput":
            if name == partition_name:
                continue
            in_names.append(name)
        elif alloc.kind == "ExternalOutput":
            out_names.append(name)
            shape = tuple(alloc.tensor_shape)
            dtp = mybir.dt.np(alloc.dtype)
            out_avals.append(jax.core.ShapedArray(shape, dtp))
            zero_outs.append(np.zeros(shape, dtp))
    n_params = len(in_names)
    all_in_names = tuple(in_names) + tuple(out_names)
    if partition_name is not None:
        all_in_names = all_in_names + (partition_name,)

    bass2jax.install_neuronx_cc_hook()
    devices = jax.devices()[:N_CORES]
    mesh = Mesh(np.asarray(devices), ("core",))

    def _body(*args):
        operands = list(args)
        if partition_name is not None:
            operands.append(bass2jax.partition_id_tensor())
        outs = bass2jax._bass_exec_p.bind(
            *operands,
            out_avals=tuple(out_avals),
            in_names=all_in_names,
            out_names=tuple(out_names),
            lowering_input_output_aliases=(),
            sim_require_finite=True,
            sim_require_nnan=True,
            nc=nc,
        )
        return tuple(outs)

    in_specs = (PartitionSpec("core"),) * (n_params + len(out_names))
    out_specs = (PartitionSpec("core"),) * len(out_names)
    jfn = jax.jit(
        shard_map(_body, mesh=mesh, in_specs=in_specs, out_specs=out_specs, check_rep=False),
        keep_unused=True,
    )
    _STATE[key] = st = dict(
        nc=nc,
        jfn=jfn,
        in_names=in_names,
        out_names=out_names,
        zero_outs=zero_outs,
        mesh=mesh,
        pspec=PartitionSpec("core"),
        jax=jax,
    )
    return st


def prepare_global_args(x, coeffs, base_weights):
    """Host prep: basis/silu precompute + global (8*shape[0], ...) concat
    arrays in the order the jitted function expects them."""
    x = np.asarray(x, dtype=np.float32)
    coeffs = np.asarray(coeffs, dtype=np.float32)
    base_weights = np.asarray(base_weights, dtype=np.float32)

    B = _bspline_basis(x)  # [4096, 6]
    sx = (x / (1.0 + np.exp(-x))).astype(np.float32)  # silu
    packed = np.concatenate(
        [B.reshape(NCHUNK, P, NB), sx.reshape(NCHUNK, P, 1)], axis=2
    )  # [32, 128, 7]
    bsx = np.ascontiguousarray(packed.transpose(1, 0, 2).reshape(P, NCHUNK * 7))

    st = _build_state()
    glob = {
        "cof": np.ascontiguousarray(
            coeffs.reshape(IN_FEAT, N_CORES, ISH, NB).transpose(1, 0, 2, 3)
        ).reshape(N_CORES * IN_FEAT, ISH, NB),
        "bw": np.ascontiguousarray(
            base_weights.reshape(IN_FEAT, N_CORES, ISH).transpose(1, 0, 2)
        ).reshape(N_CORES * IN_FEAT, ISH),
        "bsx": np.tile(bsx, (N_CORES, 1)),
    }
    args = [glob[name] for name in st["in_names"]]
    for z in st["zero_outs"]:
        args.append(np.tile(z, (N_CORES,) + (1,) * (z.ndim - 1)))
    return args


def kernel(x, coeffs, base_weights):
    st = _build_state()
    args = prepare_global_args(x, coeffs, base_weights)
    outs = st["jfn"](*args)
    out_g = np.asarray(outs[0])  # [8, 256]
    return out_g.reshape(OUT_FEAT).astype(np.float32)
